# revision 5
# baseline (speedup 1.0000x reference)
"""GCN autoencoder forward pass on 8 Trainium2 NeuronCores (Bass/Tile).

Strategy (graph/data parallel, per the sharding hint):
  - Nodes are permuted by in-degree and dealt to 8 cores tile-round-robin
    (128-node tiles -> uniform per-tile max degree, balanced edges).
  - Each conv layer: per-core matmul of its node shard (table rows
    m = dinv * (act @ W)), AllGather of the shard tables (halo exchange),
    then per-core aggregation of its tiles' in-edges with SWDGE
    indirect-DMA gather-accumulate chains (self-loop via one contiguous
    DMA, remaining in-edges 128 rows/instruction, sentinel slots read a
    zero row).
  - The 5-wide bottleneck (relu->linear->layernorm) is computed rowwise
    per tile; aggregation for the decoder's first conv happens after the
    (5->128) matmul so all gathers move 512B rows.

Self-contained: includes the walrus sync-wait compat shim and a PJRT
runner (axon) replicating bass2jax.run_bass_via_pjrt.
"""

import sys

for _p in ("/opt/trn_rl_repo",):
    if _p not in sys.path:
        sys.path.insert(0, _p)

import numpy as np

import concourse.bass as bass
import concourse.mybir as mybir
import concourse.tile as tile
from concourse.masks import make_identity
from concourse.vector_clock import ScopedClock

P = 128
NCORES = 8
N = 100000
TILES_PER_CORE = 98
SHARD = TILES_PER_CORE * P          # 12544
NPAD = NCORES * SHARD               # 100352
ZROW = NPAD                         # zero sentinel row in every table
NTILES = NCORES * TILES_PER_CORE    # 784
F1, F2, FZ, FO = 128, 64, 5, 128
EPS = 1e-5
AF = mybir.ActivationFunctionType

# ---------------------------------------------------------------- compat ----

MAX_WAITS = 1


def _split_sync_waits(nc, max_waits=MAX_WAITS):
    """This container's walrus rejects >1 sync wait per instruction; move
    excess waits onto same-engine NOPs placed just before the offender."""
    for fn in nc.m.functions:
        for bb in fn.blocks:
            out = []
            for inst in bb.instructions:
                si = inst.sync_info
                if si is not None and si.on_wait and len(si.on_wait) > max_waits:
                    waits = list(si.on_wait)
                    head, tail = waits[:-max_waits], waits[-max_waits:]
                    for i in range(0, len(head), max_waits):
                        out.append(
                            mybir.InstNoOp(
                                name=f"{inst.name}-ws{i}",
                                engine=inst.engine,
                                bass_nofuse=True,
                                sync_info=mybir.SyncInfo(
                                    on_wait=head[i : i + max_waits], on_update=[]
                                ),
                            )
                        )
                    si.on_wait = tail
                out.append(inst)
            bb.instructions[:] = out


class CompatTileContext(tile.TileContext):
    def __exit__(self, *args):
        ret = super().__exit__(*args)
        _split_sync_waits(self.nc)
        return ret


# ---------------------------------------------------------------- runner ----


class SpmdRunner:
    def __init__(self, nc, n_cores=NCORES):
        import jax
        from jax.sharding import Mesh, PartitionSpec, NamedSharding
        from jax.experimental.shard_map import shard_map
        from concourse import bass2jax
        from concourse.bass2jax import _bass_exec_p, install_neuronx_cc_hook

        install_neuronx_cc_hook()
        mybir.codegen_inst_isa_subclasses(nc)
        self.jax = jax
        self.nc = nc
        self.n_cores = n_cores
        partition_name = (
            nc.partition_id_tensor.name if nc.partition_id_tensor else None
        )

        in_names, out_names, out_avals, zero_outs = [], [], [], []
        for alloc in nc.m.functions[0].allocations:
            if not isinstance(alloc, mybir.MemoryLocationSet):
                continue
            name = alloc.memorylocations[0].name
            if alloc.kind == "ExternalInput":
                if name != partition_name:
                    in_names.append(name)
            elif alloc.kind == "ExternalOutput":
                out_names.append(name)
                shape = tuple(alloc.tensor_shape)
                dtype = mybir.dt.np(alloc.dtype)
                out_avals.append(jax.core.ShapedArray(shape, dtype))
                zero_outs.append(np.zeros(shape, dtype))
        self.in_names = in_names
        self.out_names = out_names
        self.out_avals = out_avals
        self.zero_outs = zero_outs
        n_params = len(in_names)
        all_in_names = in_names + out_names
        if partition_name is not None:
            all_in_names = all_in_names + [partition_name]

        def _body(*args):
            operands = list(args)
            if partition_name is not None:
                operands.append(bass2jax.partition_id_tensor())
            outs = _bass_exec_p.bind(
                *operands,
                out_avals=tuple(out_avals),
                in_names=tuple(all_in_names),
                out_names=tuple(out_names),
                lowering_input_output_aliases=(),
                sim_require_finite=True,
                sim_require_nnan=True,
                nc=nc,
            )
            return tuple(outs)

        devices = jax.devices()[:n_cores]
        self.mesh = Mesh(np.asarray(devices), ("core",))
        in_specs = (PartitionSpec("core"),) * (n_params + len(out_names))
        out_specs = (PartitionSpec("core"),) * len(out_names)
        self.sharding = NamedSharding(self.mesh, PartitionSpec("core"))
        self.fn = jax.jit(
            shard_map(_body, mesh=self.mesh, in_specs=in_specs,
                      out_specs=out_specs, check_rep=False),
            keep_unused=True,
        )
        self._dev_args = None

    def stage(self, in_maps):
        n = self.n_cores
        concat = [
            np.concatenate([np.asarray(in_maps[c][name]) for c in range(n)], axis=0)
            for name in self.in_names
        ]
        concat += [
            np.zeros((n * z.shape[0], *z.shape[1:]), z.dtype) for z in self.zero_outs
        ]
        self._dev_args = [self.jax.device_put(a, self.sharding) for a in concat]
        for a in self._dev_args:
            a.block_until_ready()

    def run(self):
        outs = self.fn(*self._dev_args)
        self.jax.block_until_ready(outs)
        return outs

    def results(self, outs):
        res = []
        for c in range(self.n_cores):
            d = {}
            for i, name in enumerate(self.out_names):
                full = np.asarray(outs[i])
                d[name] = full.reshape(self.n_cores, *self.out_avals[i].shape)[c]
            res.append(d)
        return res


# ------------------------------------------------------------------ plan ----


def build_plan(edge_index):
    src0 = np.asarray(edge_index[0], dtype=np.int64)
    dst0 = np.asarray(edge_index[1], dtype=np.int64)

    deg = np.bincount(dst0, minlength=N).astype(np.int64) + 1  # + self loop

    order = np.argsort(-deg, kind="stable")
    new_id = np.full(N, -1, dtype=np.int64)
    old_of_new = np.full(NPAD, -1, dtype=np.int64)
    n_data_tiles = (N + P - 1) // P  # 782
    for t in range(n_data_tiles):
        core = t % NCORES
        pos = t // NCORES
        rows = order[t * P : (t + 1) * P]
        base = core * SHARD + pos * P
        new_id[rows] = base + np.arange(len(rows))
        old_of_new[base : base + len(rows)] = rows

    nsrc = new_id[src0]
    ndst = new_id[dst0]

    order_e = np.argsort(ndst, kind="stable")
    s_sorted = nsrc[order_e]
    ptr = np.zeros(NPAD + 1, dtype=np.int64)
    np.cumsum(np.bincount(ndst, minlength=NPAD), out=ptr[1:])

    # non-self wave count per tile position (max over cores, >= 0)
    indeg = ptr[1:] - ptr[:-1]                       # without self loops
    tile_k = indeg.reshape(NTILES, P).max(axis=1)
    waves = np.zeros(TILES_PER_CORE, dtype=np.int64)
    for i in range(TILES_PER_CORE):
        waves[i] = max(tile_k[c * TILES_PER_CORE + i] for c in range(NCORES))
    tot = int(waves.sum())
    slot_off = np.concatenate([[0], np.cumsum(waves)]).astype(int)

    core_idx = []
    for c in range(NCORES):
        idx = np.full((P, max(tot, 1)), ZROW, dtype=np.int32)
        for i in range(TILES_PER_CORE):
            base = c * SHARD + i * P
            k = int(waves[i])
            for p in range(P):
                lo, hi = ptr[base + p], ptr[base + p + 1]
                m = hi - lo
                if m:
                    idx[p, slot_off[i] : slot_off[i] + m] = s_sorted[lo:hi]
        core_idx.append(idx)

    dinv = np.zeros(NPAD, dtype=np.float64)
    real = old_of_new >= 0
    dinv[real] = 1.0 / np.sqrt(deg[old_of_new[real]].astype(np.float64))
    dinv = dinv.astype(np.float32)
    dinv_cols = [
        dinv[c * SHARD : (c + 1) * SHARD].reshape(TILES_PER_CORE, P).T.copy()
        for c in range(NCORES)
    ]
    return {
        "old_of_new": old_of_new,
        "waves": [int(w) for w in waves],
        "core_idx": core_idx,
        "dinv_cols": dinv_cols,
    }


# ---------------------------------------------------------------- program ---


def build_program(waves_per_tile):
    nc = bass.Bass("TRN2", target_bir_lowering=False, debug=False,
                   enable_asserts=True, num_devices=NCORES)
    tot_slots = max(sum(waves_per_tile), 1)

    x_s = nc.dram_tensor("x_shard", [SHARD, F1], mybir.dt.float32, kind="ExternalInput").ap()
    idx = nc.dram_tensor("idx", [P, tot_slots], mybir.dt.int32, kind="ExternalInput").ap()
    dinv_c = nc.dram_tensor("dinv_cols", [P, TILES_PER_CORE], mybir.dt.float32, kind="ExternalInput").ap()
    W1e = nc.dram_tensor("W1e", [F1, F1], mybir.dt.float32, kind="ExternalInput").ap()
    W2e = nc.dram_tensor("W2e", [F1, F2], mybir.dt.float32, kind="ExternalInput").ap()
    Wm = nc.dram_tensor("Wm", [F2, FZ], mybir.dt.float32, kind="ExternalInput").ap()
    W1d = nc.dram_tensor("W1d", [FZ, F1], mybir.dt.float32, kind="ExternalInput").ap()
    W2d = nc.dram_tensor("W2d", [F1, FO], mybir.dt.float32, kind="ExternalInput").ap()
    biases = nc.dram_tensor("biases", [P, F1 + F2 + FZ + F1 + FO + 2 * FZ],
                            mybir.dt.float32, kind="ExternalInput").ap()
    out_t = nc.dram_tensor("out", [SHARD, FO], mybir.dt.float32, kind="ExternalOutput").ap()

    bounce1 = nc.dram_tensor("bounce1", [SHARD, F1], mybir.dt.float32).ap()
    bounce2 = nc.dram_tensor("bounce2", [SHARD, F2], mybir.dt.float32).ap()
    bounce3 = nc.dram_tensor("bounce3", [SHARD, F1], mybir.dt.float32).ap()
    bounce4 = nc.dram_tensor("bounce4", [SHARD, FO], mybir.dt.float32).ap()
    t1 = nc.dram_tensor("t1", [NPAD + 1, F1], mybir.dt.float32).ap()
    t2 = nc.dram_tensor("t2", [NPAD + 1, F2], mybir.dt.float32).ap()
    t3 = nc.dram_tensor("t3", [NPAD + 1, F1], mybir.dt.float32).ap()
    t4 = nc.dram_tensor("t4", [NPAD + 1, FO], mybir.dt.float32).ap()

    rg = [list(range(NCORES))]
    slot_off = np.concatenate([[0], np.cumsum(waves_per_tile)]).astype(int)
    # bias column offsets within the packed biases tensor
    OB1, OB2, OBM, OB1D, OB2D = 0, F1, F1 + F2, F1 + F2 + FZ, F1 + F2 + FZ + F1
    OLNW = OB2D + FO
    OLNB = OLNW + FZ

    with CompatTileContext(nc) as tc:
        with (
            tc.tile_pool(name="const", bufs=1) as constp,
            tc.tile_pool(name="work", bufs=3) as workp,
            tc.tile_pool(name="acc", bufs=2) as accp,
            tc.tile_pool(name="psum", bufs=2, space="PSUM") as psump,
        ):
            ident = constp.tile([P, P], mybir.dt.float32)
            make_identity(nc, ident[:])
            idx_t = constp.tile([P, tot_slots], mybir.dt.int32)
            nc.sync.dma_start(out=idx_t[:], in_=idx[:])
            dinv_t = constp.tile([P, TILES_PER_CORE], mybir.dt.float32)
            nc.sync.dma_start(out=dinv_t[:], in_=dinv_c[:])
            w1e_t = constp.tile([F1, F1], mybir.dt.float32)
            nc.sync.dma_start(out=w1e_t[:], in_=W1e[:])
            w2e_t = constp.tile([F1, F2], mybir.dt.float32)
            nc.sync.dma_start(out=w2e_t[:], in_=W2e[:])
            wm_t = constp.tile([F2, FZ], mybir.dt.float32)
            nc.sync.dma_start(out=wm_t[:], in_=Wm[:])
            w1d_t = constp.tile([FZ, F1], mybir.dt.float32)
            nc.sync.dma_start(out=w1d_t[:], in_=W1d[:])
            w2d_t = constp.tile([F1, FO], mybir.dt.float32)
            nc.sync.dma_start(out=w2d_t[:], in_=W2d[:])
            bias_t = constp.tile([P, F1 + F2 + FZ + F1 + FO + 2 * FZ], mybir.dt.float32)
            nc.sync.dma_start(out=bias_t[:], in_=biases[:])
            zero_t = constp.tile([1, F1], mybir.dt.float32)
            nc.gpsimd.memset(zero_t[:], 0.0)
            inv5_t = constp.tile([P, 1], mybir.dt.float32)
            nc.gpsimd.memset(inv5_t[:], 1.0 / FZ)
            eps_t = constp.tile([P, 1], mybir.dt.float32)
            nc.gpsimd.memset(eps_t[:], EPS)
            nc.sync.dma_start(out=t1[ZROW : ZROW + 1, :], in_=zero_t[:, :F1])
            nc.sync.dma_start(out=t2[ZROW : ZROW + 1, :], in_=zero_t[:, :F2])
            nc.sync.dma_start(out=t3[ZROW : ZROW + 1, :], in_=zero_t[:, :F1])
            nc.sync.dma_start(out=t4[ZROW : ZROW + 1, :], in_=zero_t[:, :FO])

            def produce(i, act_tile, fin, w_t, fout, dst_bounce):
                tr = psump.tile([fin, P], mybir.dt.float32, tag="tr")
                nc.tensor.transpose(out=tr[:], in_=act_tile[:, :fin], identity=ident[:])
                trs = workp.tile([fin, P], mybir.dt.float32, tag="trs")
                nc.vector.tensor_copy(out=trs[:], in_=tr[:])
                mm = psump.tile([P, fout], mybir.dt.float32, tag="mm")
                nc.tensor.matmul(mm[:], lhsT=trs[:], rhs=w_t[:, :fout], start=True, stop=True)
                ms = workp.tile([P, fout], mybir.dt.float32, tag="ms")
                nc.scalar.activation(ms[:], mm[:], AF.Copy, scale=dinv_t[:, i : i + 1])
                nc.sync.dma_start(out=dst_bounce[i * P : (i + 1) * P, :], in_=ms[:])

            def aggregate(i, table, fout, selfsrc):
                acc = accp.tile([P, fout], mybir.dt.float32, tag=f"acc{i % 14}")
                nc.sync.dma_start(out=acc[:], in_=selfsrc[i * P : (i + 1) * P, :])
                for w in range(waves_per_tile[i]):
                    col = int(slot_off[i]) + w
                    nc.gpsimd.indirect_dma_start(
                        out=acc[:],
                        out_offset=None,
                        in_=table[:],
                        in_offset=bass.IndirectOffsetOnAxis(
                            ap=idx_t[:, col : col + 1], axis=0
                        ),
                        compute_op=mybir.AluOpType.add,
                    )
                return acc

            def epilogue(i, acc, fout, bias_off, relu):
                """dinv*acc + b, optional relu -> sbuf tile"""
                e1 = workp.tile([P, fout], mybir.dt.float32, tag="epi")
                nc.scalar.activation(e1[:], acc[:], AF.Copy, scale=dinv_t[:, i : i + 1])
                e2 = workp.tile([P, fout], mybir.dt.float32, tag="epi2")
                nc.vector.tensor_add(
                    out=e2[:], in0=e1[:], in1=bias_t[:, bias_off : bias_off + fout])
                if relu:
                    e3 = workp.tile([P, fout], mybir.dt.float32, tag="epi3")
                    nc.scalar.activation(e3[:], e2[:], AF.Relu)
                    return e3
                return e2

            # ---- L1 produce
            for i in range(TILES_PER_CORE):
                xa = workp.tile([P, F1], mybir.dt.float32, tag="xa")
                nc.sync.dma_start(out=xa[:], in_=x_s[i * P : (i + 1) * P, :])
                produce(i, xa, F1, w1e_t, F1, bounce1)
            nc.gpsimd.collective_compute(
                "AllGather", mybir.AluOpType.bypass, replica_groups=rg,
                ins=[bounce1[:]], outs=[t1[0:NPAD, :]])

            # ---- L1 aggregate -> h (relu) -> L2 produce
            for i in range(TILES_PER_CORE):
                acc = aggregate(i, t1, F1, bounce1)
                h = epilogue(i, acc, F1, OB1, relu=True)
                produce(i, h, F1, w2e_t, F2, bounce2)
            nc.gpsimd.collective_compute(
                "AllGather", mybir.AluOpType.bypass, replica_groups=rg,
                ins=[bounce2[:]], outs=[t2[0:NPAD, :]])

            # ---- L2 aggregate -> z -> bottleneck -> L3 produce
            for i in range(TILES_PER_CORE):
                acc2 = aggregate(i, t2, F2, bounce2)
                z = epilogue(i, acc2, F2, OB2, relu=False)
                zr = workp.tile([P, F2], mybir.dt.float32, tag="zrl")
                nc.scalar.activation(zr[:], z[:], AF.Relu)
                tr2 = psump.tile([F2, P], mybir.dt.float32, tag="tr")
                nc.tensor.transpose(out=tr2[:], in_=zr[:], identity=ident[:])
                tr2s = workp.tile([F2, P], mybir.dt.float32, tag="trs")
                nc.vector.tensor_copy(out=tr2s[:], in_=tr2[:])
                zm = psump.tile([P, FZ], mybir.dt.float32, tag="mm")
                nc.tensor.matmul(zm[:], lhsT=tr2s[:], rhs=wm_t[:], start=True, stop=True)
                zms = workp.tile([P, FZ], mybir.dt.float32, tag="zms")
                nc.vector.tensor_add(
                    out=zms[:], in0=zm[:], in1=bias_t[:, OBM : OBM + FZ])
                musum = workp.tile([P, 1], mybir.dt.float32, tag="musum")
                nc.vector.reduce_sum(musum[:], zms[:], axis=mybir.AxisListType.X)
                mu = workp.tile([P, 1], mybir.dt.float32, tag="mu")
                nc.vector.tensor_mul(out=mu[:], in0=musum[:], in1=inv5_t[:])
                diff = workp.tile([P, FZ], mybir.dt.float32, tag="diff")
                nc.vector.tensor_tensor(
                    out=diff[:], in0=zms[:], in1=mu[:].to_broadcast([P, FZ]),
                    op=mybir.AluOpType.subtract)
                sq = workp.tile([P, FZ], mybir.dt.float32, tag="sq")
                nc.vector.tensor_mul(out=sq[:], in0=diff[:], in1=diff[:])
                varsum = workp.tile([P, 1], mybir.dt.float32, tag="varsum")
                nc.vector.reduce_sum(varsum[:], sq[:], axis=mybir.AxisListType.X)
                var = workp.tile([P, 1], mybir.dt.float32, tag="var")
                nc.vector.tensor_mul(out=var[:], in0=varsum[:], in1=inv5_t[:])
                vare = workp.tile([P, 1], mybir.dt.float32, tag="vare")
                nc.vector.tensor_add(out=vare[:], in0=var[:], in1=eps_t[:])
                sd = workp.tile([P, 1], mybir.dt.float32, tag="sd")
                nc.scalar.activation(sd[:], vare[:], AF.Sqrt)
                rinv = workp.tile([P, 1], mybir.dt.float32, tag="rinv")
                nc.vector.reciprocal(rinv[:], sd[:])
                zn = workp.tile([P, FZ], mybir.dt.float32, tag="zn")
                nc.vector.tensor_mul(out=zn[:], in0=diff[:], in1=rinv[:].to_broadcast([P, FZ]))
                zw = workp.tile([P, FZ], mybir.dt.float32, tag="zw")
                nc.vector.tensor_mul(out=zw[:], in0=zn[:], in1=bias_t[:, OLNW : OLNW + FZ])
                zl = workp.tile([P, FZ], mybir.dt.float32, tag="zl")
                nc.vector.tensor_add(out=zl[:], in0=zw[:], in1=bias_t[:, OLNB : OLNB + FZ])
                produce(i, zl, FZ, w1d_t, F1, bounce3)
            nc.gpsimd.collective_compute(
                "AllGather", mybir.AluOpType.bypass, replica_groups=rg,
                ins=[bounce3[:]], outs=[t3[0:NPAD, :]])

            # ---- L3 aggregate -> d (relu) -> L4 produce
            for i in range(TILES_PER_CORE):
                acc3 = aggregate(i, t3, F1, bounce3)
                d = epilogue(i, acc3, F1, OB1D, relu=True)
                produce(i, d, F1, w2d_t, FO, bounce4)
            nc.gpsimd.collective_compute(
                "AllGather", mybir.AluOpType.bypass, replica_groups=rg,
                ins=[bounce4[:]], outs=[t4[0:NPAD, :]])

            # ---- L4 aggregate -> output
            for i in range(TILES_PER_CORE):
                acc4 = aggregate(i, t4, FO, bounce4)
                o = epilogue(i, acc4, FO, OB2D, relu=False)
                nc.sync.dma_start(out=out_t[i * P : (i + 1) * P, :], in_=o[:])
    return nc


# ------------------------------------------------------------------ kernel --

_CACHE = {}


def kernel(x, edge_index, W1e, b1e, W2e, b2e, Wm, bm, ln_w, ln_b,
           W1d, b1d, W2d, b2d):
    x = np.asarray(x, dtype=np.float32)
    edge_index = np.asarray(edge_index)
    plan = build_plan(edge_index)
    old_of_new = plan["old_of_new"]
    real = old_of_new >= 0

    # pack per-core inputs
    xg = np.zeros((NPAD, F1), np.float32)
    xg[real] = x[old_of_new[real]]
    bias_pack = np.zeros((P, F1 + F2 + FZ + F1 + FO + 2 * FZ), np.float32)
    o = 0
    for vec in (b1e, b2e, bm, b1d, b2d, ln_w, ln_b):
        v = np.asarray(vec, np.float32).ravel()
        bias_pack[:, o : o + v.size] = v[None, :]
        o += v.size

    in_maps = []
    for c in range(NCORES):
        in_maps.append({
            "x_shard": xg[c * SHARD : (c + 1) * SHARD],
            "idx": plan["core_idx"][c],
            "dinv_cols": plan["dinv_cols"][c],
            "W1e": np.asarray(W1e, np.float32),
            "W2e": np.asarray(W2e, np.float32),
            "Wm": np.asarray(Wm, np.float32),
            "W1d": np.asarray(W1d, np.float32),
            "W2d": np.asarray(W2d, np.float32),
            "biases": bias_pack,
        })

    key = tuple(plan["waves"])
    if key not in _CACHE:
        nc = build_program(plan["waves"])
        _CACHE[key] = SpmdRunner(nc)
    runner = _CACHE[key]
    runner.stage(in_maps)
    res = runner.results(runner.run())

    out_new = np.concatenate([res[c]["out"] for c in range(NCORES)], axis=0)
    out = np.zeros((N, FO), np.float32)
    out[old_of_new[real]] = out_new[real]
    return out


# revision 11
# speedup vs baseline: 4.0106x; 4.0106x over previous
"""GCN autoencoder forward pass on 8 Trainium2 NeuronCores (Bass/Tile).

Strategy (graph/data parallel, per the sharding hint):
  - Nodes are permuted by in-degree and dealt to 8 cores tile-round-robin
    (128-node tiles -> uniform per-tile max degree, balanced edges).
  - Each conv layer: per-core matmul of its node shard (table rows
    m = dinv * (act @ W)), AllGather of the shard tables (halo exchange),
    then per-core aggregation of its tiles' in-edges with SWDGE
    indirect-DMA gather-accumulate chains (self-loop via one contiguous
    DMA, remaining in-edges 128 rows/instruction, sentinel slots read a
    zero row).
  - The 5-wide bottleneck (relu->linear->layernorm) is computed rowwise
    per tile; aggregation for the decoder's first conv happens after the
    (5->128) matmul so all gathers move 512B rows.

Self-contained: includes the walrus sync-wait compat shim and a PJRT
runner (axon) replicating bass2jax.run_bass_via_pjrt.
"""

import sys

for _p in ("/opt/trn_rl_repo",):
    if _p not in sys.path:
        sys.path.insert(0, _p)

import numpy as np

import concourse.bass as bass
import concourse.mybir as mybir
import concourse.tile as tile
from concourse.masks import make_identity
from concourse.vector_clock import ScopedClock

P = 128
NCORES = 8
N = 100000
TILES_PER_CORE = 98
SHARD = TILES_PER_CORE * P          # 12544
NPAD = NCORES * SHARD               # 100352
ZROW = NPAD                         # zero sentinel row in every table
NTILES = NCORES * TILES_PER_CORE    # 784
F1, F2, FZ, FO = 128, 64, 5, 128
EPS = 1e-5
AF = mybir.ActivationFunctionType

# ---------------------------------------------------------------- compat ----

MAX_WAITS = 1


def _split_sync_waits(nc, max_waits=MAX_WAITS):
    """This container's walrus rejects >1 sync wait per instruction; move
    excess waits onto same-engine NOPs placed just before the offender."""
    for fn in nc.m.functions:
        for bb in fn.blocks:
            out = []
            for inst in bb.instructions:
                si = inst.sync_info
                if si is not None and si.on_wait and len(si.on_wait) > max_waits:
                    waits = list(si.on_wait)
                    head, tail = waits[:-max_waits], waits[-max_waits:]
                    for i in range(0, len(head), max_waits):
                        out.append(
                            mybir.InstNoOp(
                                name=f"{inst.name}-ws{i}",
                                engine=inst.engine,
                                bass_nofuse=True,
                                sync_info=mybir.SyncInfo(
                                    on_wait=head[i : i + max_waits], on_update=[]
                                ),
                            )
                        )
                    si.on_wait = tail
                out.append(inst)
            bb.instructions[:] = out


class CompatTileContext(tile.TileContext):
    def __exit__(self, *args):
        ret = super().__exit__(*args)
        _split_sync_waits(self.nc)
        return ret


# ---------------------------------------------------------------- runner ----


class SpmdRunner:
    def __init__(self, nc, n_cores=NCORES):
        import jax
        from jax.sharding import Mesh, PartitionSpec, NamedSharding
        from jax.experimental.shard_map import shard_map
        from concourse import bass2jax
        from concourse.bass2jax import _bass_exec_p, install_neuronx_cc_hook

        install_neuronx_cc_hook()
        mybir.codegen_inst_isa_subclasses(nc)
        self.jax = jax
        self.nc = nc
        self.n_cores = n_cores
        partition_name = (
            nc.partition_id_tensor.name if nc.partition_id_tensor else None
        )

        in_names, out_names, out_avals, zero_outs = [], [], [], []
        for alloc in nc.m.functions[0].allocations:
            if not isinstance(alloc, mybir.MemoryLocationSet):
                continue
            name = alloc.memorylocations[0].name
            if alloc.kind == "ExternalInput":
                if name != partition_name:
                    in_names.append(name)
            elif alloc.kind == "ExternalOutput":
                out_names.append(name)
                shape = tuple(alloc.tensor_shape)
                dtype = mybir.dt.np(alloc.dtype)
                out_avals.append(jax.core.ShapedArray(shape, dtype))
                zero_outs.append(np.zeros(shape, dtype))
        self.in_names = in_names
        self.out_names = out_names
        self.out_avals = out_avals
        self.zero_outs = zero_outs
        n_params = len(in_names)
        all_in_names = in_names + out_names
        if partition_name is not None:
            all_in_names = all_in_names + [partition_name]

        def _body(*args):
            operands = list(args)
            if partition_name is not None:
                operands.append(bass2jax.partition_id_tensor())
            outs = _bass_exec_p.bind(
                *operands,
                out_avals=tuple(out_avals),
                in_names=tuple(all_in_names),
                out_names=tuple(out_names),
                lowering_input_output_aliases=(),
                sim_require_finite=True,
                sim_require_nnan=True,
                nc=nc,
            )
            return tuple(outs)

        devices = jax.devices()[:n_cores]
        self.mesh = Mesh(np.asarray(devices), ("core",))
        in_specs = (PartitionSpec("core"),) * (n_params + len(out_names))
        out_specs = (PartitionSpec("core"),) * len(out_names)
        self.sharding = NamedSharding(self.mesh, PartitionSpec("core"))
        self.fn = jax.jit(
            shard_map(_body, mesh=self.mesh, in_specs=in_specs,
                      out_specs=out_specs, check_rep=False),
            keep_unused=True,
        )
        self._dev_args = None

    def stage(self, in_maps):
        n = self.n_cores
        concat = [
            np.concatenate([np.asarray(in_maps[c][name]) for c in range(n)], axis=0)
            for name in self.in_names
        ]
        concat += [
            np.zeros((n * z.shape[0], *z.shape[1:]), z.dtype) for z in self.zero_outs
        ]
        self._dev_args = [self.jax.device_put(a, self.sharding) for a in concat]
        for a in self._dev_args:
            a.block_until_ready()

    def run(self):
        outs = self.fn(*self._dev_args)
        self.jax.block_until_ready(outs)
        return outs

    def results(self, outs):
        res = []
        for c in range(self.n_cores):
            d = {}
            for i, name in enumerate(self.out_names):
                full = np.asarray(outs[i])
                d[name] = full.reshape(self.n_cores, *self.out_avals[i].shape)[c]
            res.append(d)
        return res


# ------------------------------------------------------------------ plan ----


def build_plan(edge_index):
    src0 = np.asarray(edge_index[0], dtype=np.int64)
    dst0 = np.asarray(edge_index[1], dtype=np.int64)

    deg = np.bincount(dst0, minlength=N).astype(np.int64) + 1  # + self loop

    order = np.argsort(-deg, kind="stable")
    new_id = np.full(N, -1, dtype=np.int64)
    old_of_new = np.full(NPAD, -1, dtype=np.int64)
    n_data_tiles = (N + P - 1) // P  # 782
    for t in range(n_data_tiles):
        core = t % NCORES
        pos = t // NCORES
        rows = order[t * P : (t + 1) * P]
        base = core * SHARD + pos * P
        new_id[rows] = base + np.arange(len(rows))
        old_of_new[base : base + len(rows)] = rows

    nsrc = new_id[src0]
    ndst = new_id[dst0]

    order_e = np.argsort(ndst, kind="stable")
    s_sorted = nsrc[order_e]
    ptr = np.zeros(NPAD + 1, dtype=np.int64)
    np.cumsum(np.bincount(ndst, minlength=NPAD), out=ptr[1:])

    # non-self wave count per tile position (max over cores, >= 0)
    indeg = ptr[1:] - ptr[:-1]                       # without self loops
    tile_k = indeg.reshape(NTILES, P).max(axis=1)
    waves = np.zeros(TILES_PER_CORE, dtype=np.int64)
    for i in range(TILES_PER_CORE):
        waves[i] = max(tile_k[c * TILES_PER_CORE + i] for c in range(NCORES))
    tot = int(waves.sum())
    slot_off = np.concatenate([[0], np.cumsum(waves)]).astype(int)

    core_idx = []
    for c in range(NCORES):
        idx = np.full((P, max(tot, 1)), ZROW, dtype=np.int32)
        for i in range(TILES_PER_CORE):
            base = c * SHARD + i * P
            k = int(waves[i])
            for p in range(P):
                lo, hi = ptr[base + p], ptr[base + p + 1]
                m = hi - lo
                if m:
                    idx[p, slot_off[i] : slot_off[i] + m] = s_sorted[lo:hi]
        core_idx.append(idx)

    dinv = np.zeros(NPAD, dtype=np.float64)
    real = old_of_new >= 0
    dinv[real] = 1.0 / np.sqrt(deg[old_of_new[real]].astype(np.float64))
    dinv = dinv.astype(np.float32)
    dinv_cols = [
        dinv[c * SHARD : (c + 1) * SHARD].reshape(TILES_PER_CORE, P).T.copy()
        for c in range(NCORES)
    ]
    return {
        "old_of_new": old_of_new,
        "waves": [int(w) for w in waves],
        "core_idx": core_idx,
        "dinv_cols": dinv_cols,
    }


# ---------------------------------------------------------------- program ---


def build_program(waves_per_tile):
    nc = bass.Bass("TRN2", target_bir_lowering=False, debug=False,
                   enable_asserts=True, num_devices=NCORES)
    tot_slots = max(sum(waves_per_tile), 1)

    x_s = nc.dram_tensor("x_shard", [SHARD, F1], mybir.dt.float32, kind="ExternalInput").ap()
    idx = nc.dram_tensor("idx", [P, tot_slots], mybir.dt.int32, kind="ExternalInput").ap()
    dinv_c = nc.dram_tensor("dinv_cols", [P, TILES_PER_CORE], mybir.dt.float32, kind="ExternalInput").ap()
    W1e = nc.dram_tensor("W1e", [F1, F1], mybir.dt.float32, kind="ExternalInput").ap()
    W2e = nc.dram_tensor("W2e", [F1, F2], mybir.dt.float32, kind="ExternalInput").ap()
    Wm = nc.dram_tensor("Wm", [F2, FZ], mybir.dt.float32, kind="ExternalInput").ap()
    W1d = nc.dram_tensor("W1d", [FZ, F1], mybir.dt.float32, kind="ExternalInput").ap()
    W2d = nc.dram_tensor("W2d", [F1, FO], mybir.dt.float32, kind="ExternalInput").ap()
    biases = nc.dram_tensor("biases", [P, F1 + F2 + FZ + F1 + FO + 2 * FZ],
                            mybir.dt.float32, kind="ExternalInput").ap()
    out_t = nc.dram_tensor("out", [SHARD, FO], mybir.dt.float32, kind="ExternalOutput").ap()

    bounce1 = nc.dram_tensor("bounce1", [SHARD, F1], mybir.dt.float32).ap()
    bounce2 = nc.dram_tensor("bounce2", [SHARD, F2], mybir.dt.float32).ap()
    bounce3 = nc.dram_tensor("bounce3", [SHARD, F1], mybir.dt.float32).ap()
    bounce4 = nc.dram_tensor("bounce4", [SHARD, FO], mybir.dt.float32).ap()
    t1 = nc.dram_tensor("t1", [NPAD + 1, F1], mybir.dt.float32).ap()
    t2 = nc.dram_tensor("t2", [NPAD + 1, F2], mybir.dt.float32).ap()
    t3 = nc.dram_tensor("t3", [NPAD + 1, F1], mybir.dt.float32).ap()
    t4 = nc.dram_tensor("t4", [NPAD + 1, FO], mybir.dt.float32).ap()

    rg = [list(range(NCORES))]
    slot_off = np.concatenate([[0], np.cumsum(waves_per_tile)]).astype(int)
    # bias column offsets within the packed biases tensor
    OB1, OB2, OBM, OB1D, OB2D = 0, F1, F1 + F2, F1 + F2 + FZ, F1 + F2 + FZ + F1
    OLNW = OB2D + FO
    OLNB = OLNW + FZ

    with CompatTileContext(nc) as tc:
        with (
            tc.tile_pool(name="const", bufs=1) as constp,
            tc.tile_pool(name="work", bufs=3) as workp,
            tc.tile_pool(name="acc", bufs=2) as accp,
            tc.tile_pool(name="psum", bufs=2, space="PSUM") as psump,
        ):
            ident = constp.tile([P, P], mybir.dt.float32)
            make_identity(nc, ident[:])
            idx_t = constp.tile([P, tot_slots], mybir.dt.int32)
            nc.sync.dma_start(out=idx_t[:], in_=idx[:])
            dinv_t = constp.tile([P, TILES_PER_CORE], mybir.dt.float32)
            nc.sync.dma_start(out=dinv_t[:], in_=dinv_c[:])
            w1e_t = constp.tile([F1, F1], mybir.dt.float32)
            nc.sync.dma_start(out=w1e_t[:], in_=W1e[:])
            w2e_t = constp.tile([F1, F2], mybir.dt.float32)
            nc.sync.dma_start(out=w2e_t[:], in_=W2e[:])
            wm_t = constp.tile([F2, FZ], mybir.dt.float32)
            nc.sync.dma_start(out=wm_t[:], in_=Wm[:])
            w1d_t = constp.tile([FZ, F1], mybir.dt.float32)
            nc.sync.dma_start(out=w1d_t[:], in_=W1d[:])
            w2d_t = constp.tile([F1, FO], mybir.dt.float32)
            nc.sync.dma_start(out=w2d_t[:], in_=W2d[:])
            bias_t = constp.tile([P, F1 + F2 + FZ + F1 + FO + 2 * FZ], mybir.dt.float32)
            nc.sync.dma_start(out=bias_t[:], in_=biases[:])
            zero_t = constp.tile([1, F1], mybir.dt.float32)
            nc.gpsimd.memset(zero_t[:], 0.0)
            inv5_t = constp.tile([P, 1], mybir.dt.float32)
            nc.gpsimd.memset(inv5_t[:], 1.0 / FZ)
            eps_t = constp.tile([P, 1], mybir.dt.float32)
            nc.gpsimd.memset(eps_t[:], EPS)
            nc.sync.dma_start(out=t1[ZROW : ZROW + 1, :], in_=zero_t[:, :F1])
            nc.sync.dma_start(out=t2[ZROW : ZROW + 1, :], in_=zero_t[:, :F2])
            nc.sync.dma_start(out=t3[ZROW : ZROW + 1, :], in_=zero_t[:, :F1])
            nc.sync.dma_start(out=t4[ZROW : ZROW + 1, :], in_=zero_t[:, :FO])

            def produce(i, act_tile, fin, w_t, fout, dst_bounce):
                tr = psump.tile([fin, P], mybir.dt.float32, tag="tr")
                nc.tensor.transpose(out=tr[:], in_=act_tile[:, :fin], identity=ident[:])
                trs = workp.tile([fin, P], mybir.dt.float32, tag="trs")
                nc.vector.tensor_copy(out=trs[:], in_=tr[:])
                mm = psump.tile([P, fout], mybir.dt.float32, tag="mm")
                nc.tensor.matmul(mm[:], lhsT=trs[:], rhs=w_t[:, :fout], start=True, stop=True)
                ms = workp.tile([P, fout], mybir.dt.float32, tag="ms")
                nc.scalar.activation(ms[:], mm[:], AF.Copy, scale=dinv_t[:, i : i + 1])
                nc.sync.dma_start(out=dst_bounce[i * P : (i + 1) * P, :], in_=ms[:])

            BLK = 14

            def agg_blocks(table, fout, selfsrc, finish):
                """Aggregate all tiles in blocks of BLK with waves emitted
                round-robin across the block (keeps many chains in flight on
                the POOL engine); call finish(i, acc) per tile afterwards."""
                for b0 in range(0, TILES_PER_CORE, BLK):
                    blk = range(b0, min(b0 + BLK, TILES_PER_CORE))
                    accs = {}
                    for i in blk:
                        acc = accp.tile([P, fout], mybir.dt.float32,
                                        tag=f"acc{i % BLK}")
                        nc.sync.dma_start(
                            out=acc[:], in_=selfsrc[i * P : (i + 1) * P, :])
                        accs[i] = acc
                    wmax = max(waves_per_tile[i] for i in blk)
                    for w in range(wmax):
                        for i in blk:
                            if w < waves_per_tile[i]:
                                col = int(slot_off[i]) + w
                                nc.gpsimd.indirect_dma_start(
                                    out=accs[i][:],
                                    out_offset=None,
                                    in_=table[:],
                                    in_offset=bass.IndirectOffsetOnAxis(
                                        ap=idx_t[:, col : col + 1], axis=0
                                    ),
                                    compute_op=mybir.AluOpType.add,
                                )
                    for i in blk:
                        finish(i, accs[i])

            def epilogue(i, acc, fout, bias_off, relu):
                """dinv*acc + b, optional relu -> sbuf tile"""
                e1 = workp.tile([P, fout], mybir.dt.float32, tag="epi")
                nc.scalar.activation(e1[:], acc[:], AF.Copy, scale=dinv_t[:, i : i + 1])
                e2 = workp.tile([P, fout], mybir.dt.float32, tag="epi2")
                nc.vector.tensor_add(
                    out=e2[:], in0=e1[:], in1=bias_t[:, bias_off : bias_off + fout])
                if relu:
                    e3 = workp.tile([P, fout], mybir.dt.float32, tag="epi3")
                    nc.scalar.activation(e3[:], e2[:], AF.Relu)
                    return e3
                return e2

            # ---- L1 produce
            for i in range(TILES_PER_CORE):
                xa = workp.tile([P, F1], mybir.dt.float32, tag="xa")
                nc.sync.dma_start(out=xa[:], in_=x_s[i * P : (i + 1) * P, :])
                produce(i, xa, F1, w1e_t, F1, bounce1)
            nc.gpsimd.collective_compute(
                "AllGather", mybir.AluOpType.bypass, replica_groups=rg,
                ins=[bounce1[:]], outs=[t1[0:NPAD, :]])

            # ---- L1 aggregate -> h (relu) -> L2 produce
            def fin1(i, acc):
                h = epilogue(i, acc, F1, OB1, relu=True)
                produce(i, h, F1, w2e_t, F2, bounce2)

            agg_blocks(t1, F1, bounce1, fin1)
            nc.gpsimd.collective_compute(
                "AllGather", mybir.AluOpType.bypass, replica_groups=rg,
                ins=[bounce2[:]], outs=[t2[0:NPAD, :]])

            # ---- L2 aggregate -> z -> bottleneck -> L3 produce
            def fin2(i, acc2):
                z = epilogue(i, acc2, F2, OB2, relu=False)
                zr = workp.tile([P, F2], mybir.dt.float32, tag="zrl")
                nc.scalar.activation(zr[:], z[:], AF.Relu)
                tr2 = psump.tile([F2, P], mybir.dt.float32, tag="tr")
                nc.tensor.transpose(out=tr2[:], in_=zr[:], identity=ident[:])
                tr2s = workp.tile([F2, P], mybir.dt.float32, tag="trs")
                nc.vector.tensor_copy(out=tr2s[:], in_=tr2[:])
                zm = psump.tile([P, FZ], mybir.dt.float32, tag="mm")
                nc.tensor.matmul(zm[:], lhsT=tr2s[:], rhs=wm_t[:], start=True, stop=True)
                zms = workp.tile([P, FZ], mybir.dt.float32, tag="zms")
                nc.vector.tensor_add(
                    out=zms[:], in0=zm[:], in1=bias_t[:, OBM : OBM + FZ])
                musum = workp.tile([P, 1], mybir.dt.float32, tag="musum")
                nc.vector.reduce_sum(musum[:], zms[:], axis=mybir.AxisListType.X)
                mu = workp.tile([P, 1], mybir.dt.float32, tag="mu")
                nc.vector.tensor_mul(out=mu[:], in0=musum[:], in1=inv5_t[:])
                diff = workp.tile([P, FZ], mybir.dt.float32, tag="diff")
                nc.vector.tensor_tensor(
                    out=diff[:], in0=zms[:], in1=mu[:].to_broadcast([P, FZ]),
                    op=mybir.AluOpType.subtract)
                sq = workp.tile([P, FZ], mybir.dt.float32, tag="sq")
                nc.vector.tensor_mul(out=sq[:], in0=diff[:], in1=diff[:])
                varsum = workp.tile([P, 1], mybir.dt.float32, tag="varsum")
                nc.vector.reduce_sum(varsum[:], sq[:], axis=mybir.AxisListType.X)
                var = workp.tile([P, 1], mybir.dt.float32, tag="var")
                nc.vector.tensor_mul(out=var[:], in0=varsum[:], in1=inv5_t[:])
                vare = workp.tile([P, 1], mybir.dt.float32, tag="vare")
                nc.vector.tensor_add(out=vare[:], in0=var[:], in1=eps_t[:])
                sd = workp.tile([P, 1], mybir.dt.float32, tag="sd")
                nc.scalar.activation(sd[:], vare[:], AF.Sqrt)
                rinv = workp.tile([P, 1], mybir.dt.float32, tag="rinv")
                nc.vector.reciprocal(rinv[:], sd[:])
                zn = workp.tile([P, FZ], mybir.dt.float32, tag="zn")
                nc.vector.tensor_mul(out=zn[:], in0=diff[:], in1=rinv[:].to_broadcast([P, FZ]))
                zw = workp.tile([P, FZ], mybir.dt.float32, tag="zw")
                nc.vector.tensor_mul(out=zw[:], in0=zn[:], in1=bias_t[:, OLNW : OLNW + FZ])
                zl = workp.tile([P, FZ], mybir.dt.float32, tag="zl")
                nc.vector.tensor_add(out=zl[:], in0=zw[:], in1=bias_t[:, OLNB : OLNB + FZ])
                produce(i, zl, FZ, w1d_t, F1, bounce3)

            agg_blocks(t2, F2, bounce2, fin2)
            nc.gpsimd.collective_compute(
                "AllGather", mybir.AluOpType.bypass, replica_groups=rg,
                ins=[bounce3[:]], outs=[t3[0:NPAD, :]])

            # ---- L3 aggregate -> d (relu) -> L4 produce
            def fin3(i, acc3):
                d = epilogue(i, acc3, F1, OB1D, relu=True)
                produce(i, d, F1, w2d_t, FO, bounce4)

            agg_blocks(t3, F1, bounce3, fin3)
            nc.gpsimd.collective_compute(
                "AllGather", mybir.AluOpType.bypass, replica_groups=rg,
                ins=[bounce4[:]], outs=[t4[0:NPAD, :]])

            # ---- L4 aggregate -> output
            def fin4(i, acc4):
                o = epilogue(i, acc4, FO, OB2D, relu=False)
                nc.sync.dma_start(out=out_t[i * P : (i + 1) * P, :], in_=o[:])

            agg_blocks(t4, FO, bounce4, fin4)
    return nc


# ------------------------------------------------------------------ kernel --

_CACHE = {}


def kernel(x, edge_index, W1e, b1e, W2e, b2e, Wm, bm, ln_w, ln_b,
           W1d, b1d, W2d, b2d):
    x = np.asarray(x, dtype=np.float32)
    edge_index = np.asarray(edge_index)
    plan = build_plan(edge_index)
    old_of_new = plan["old_of_new"]
    real = old_of_new >= 0

    # pack per-core inputs
    xg = np.zeros((NPAD, F1), np.float32)
    xg[real] = x[old_of_new[real]]
    bias_pack = np.zeros((P, F1 + F2 + FZ + F1 + FO + 2 * FZ), np.float32)
    o = 0
    for vec in (b1e, b2e, bm, b1d, b2d, ln_w, ln_b):
        v = np.asarray(vec, np.float32).ravel()
        bias_pack[:, o : o + v.size] = v[None, :]
        o += v.size

    in_maps = []
    for c in range(NCORES):
        in_maps.append({
            "x_shard": xg[c * SHARD : (c + 1) * SHARD],
            "idx": plan["core_idx"][c],
            "dinv_cols": plan["dinv_cols"][c],
            "W1e": np.asarray(W1e, np.float32),
            "W2e": np.asarray(W2e, np.float32),
            "Wm": np.asarray(Wm, np.float32),
            "W1d": np.asarray(W1d, np.float32),
            "W2d": np.asarray(W2d, np.float32),
            "biases": bias_pack,
        })

    key = tuple(plan["waves"])
    if key not in _CACHE:
        nc = build_program(plan["waves"])
        _CACHE[key] = SpmdRunner(nc)
    runner = _CACHE[key]
    runner.stage(in_maps)
    res = runner.results(runner.run())

    out_new = np.concatenate([res[c]["out"] for c in range(NCORES)], axis=0)
    out = np.zeros((N, FO), np.float32)
    out[old_of_new[real]] = out_new[real]
    return out
